# revision 4
# baseline (speedup 1.0000x reference)
"""Trainium2 Bass kernel for nn_BB_loss_80298708566608 (retrieval_knn).

Reference computation: for each of B*N query patches (3x3x3 = 27-dim), find the
nearest candidate patch among G=14460 database patches (built from `tar` at 3
scales with shifted grids), under l = 0.5*||tpf-g||^2 + 0.5*||ipf-g||^2.
Since the query-norm term is constant in g, argmin_g l == argmax_g score with
  score(q, g) = (tpf_q + ipf_q) . g - ||g||^2
which is one [4608, 28] x [28, G] matmul (augmented with a constant-1 column
against -||g||^2) fused with an argmax over the candidate axis.

Device strategy (8 NeuronCores, candidate-sharded):
  - core k owns candidates [k*1808, k*1808+1808) (last core 1804), padded to a
    local 2048 with score -1e30, laid out as 4 matmul chunks of 512.
  - every core processes all 36 query tiles of 128 queries; per tile the scores
    [128, 2048] live entirely in PSUM (4 banks, double-buffered), then DVE does
    reduce_max -> broadcast -> max_index (first-occurrence argmax semantics,
    matching jnp.argmin tie-breaking).
  - outputs per core: local max value + local argmax per query; the host merges
    the 8 partial argmaxes (ascending core order, strict >, preserving global
    first-tie-break), gathers the selected patches, and computes the L1 loss
    and the folded image.

The patch database is built on the host in numpy; it is bit-identical to the
reference's jax implementation (validated).
"""

import numpy as np

import concourse.bass as bass
import concourse.mybir as mybir
from concourse.tile import TileContext
from concourse.bass_utils import run_bass_kernel_spmd
from concourse.vector_clock import ScopedClock

# ---------------------------------------------------------------- constants
PS = 3
A_CUBIC = -0.75
B, C, H, W = 2, 3, 144, 144
N = (H // PS) * (W // PS)  # 2304 queries per batch
D = C * PS * PS  # 27
DA = D + 1  # augmented with constant-1 column
G = 14460  # database size per batch
NCORES = 8
G_LOC = 1808  # candidates per core (last core: 1804 valid)
CHUNK = 512
NCHUNK = 4
GL_PAD = CHUNK * NCHUNK  # 2048
NQ = B * N  # 4608 total queries
QT = 128  # queries per tile
NTILE = NQ // QT  # 36
NEG = -1.0e30

_CACHED_NC = None


# ------------------------------------------------------------- database build
def _cubic_w(t):
    at = np.abs(t)
    w1 = ((A_CUBIC + 2.0) * at - (A_CUBIC + 3.0)) * at * at + 1.0
    w2 = A_CUBIC * (((at - 5.0) * at + 8.0) * at - 4.0)
    return np.where(at <= 1.0, w1, np.where(at < 2.0, w2, 0.0)).astype(t.dtype)


def _resize_axis(x, out_size, axis):
    in_size = x.shape[axis]
    scale = (in_size - 1) / (out_size - 1)
    coords = np.arange(out_size, dtype=x.dtype) * np.asarray(scale, x.dtype)
    i0 = np.floor(coords)
    frac = coords - i0
    offs = np.arange(-1, 3)
    idx = np.clip(i0.astype(np.int32)[:, None] + offs[None, :], 0, in_size - 1)
    w = _cubic_w(frac[:, None] - offs[None, :].astype(x.dtype))
    xm = np.moveaxis(x, axis, -1)
    vals = xm[..., idx]
    out = np.einsum("...ot,ot->...o", vals, w)
    return np.moveaxis(out, -1, axis).astype(x.dtype)


def _bicubic(x, out_h, out_w):
    return _resize_axis(_resize_axis(x, out_h, 2), out_w, 3)


def _unfold(x, ps=PS):
    b, c, h, w = x.shape
    x = x.reshape(b, c, h // ps, ps, w // ps, ps)
    x = x.transpose(0, 2, 4, 1, 3, 5)
    return x.reshape(b, -1, c, ps, ps)


def _fold(p, c, h, w, ps=PS):
    b = p.shape[0]
    x = p.reshape(b, h // ps, w // ps, c, ps, ps)
    x = x.transpose(0, 3, 1, 4, 2, 5)
    return x.reshape(b, c, h, w)


def _g_database(tar, ps=PS):
    b, c, h, w = tar.shape
    scales = [tar, _bicubic(tar, h // 2, w // 2), _bicubic(tar, h // 4, w // 4)]
    patches = []
    for i in range(1, ps):
        for j in range(1, ps):
            for s in scales:
                sh, sw = s.shape[2], s.shape[3]
                if ps < min(sh, sw):
                    patches.append(
                        _unfold(s[:, :, i : sh - (ps - i), j : sw - (ps - j)], ps)
                    )
    for s in scales:
        patches.append(_unfold(s, ps))
    return np.concatenate(patches, axis=1)


# ------------------------------------------------------------------ bass IR
MAX_WAITS = 1


class SplitDrainTileContext(TileContext):
    """This walrus build rejects instructions carrying more than a couple of
    sync waits; split the tail-drain waits across extra SP nops (all before the
    all-engine barrier, so semantics are unchanged)."""

    def _drain_and_barrier(self, tick_clock, wait_clock):
        drain_inst = self.nc.sync.drain()
        wait_clock.add_sem_waits(
            drain_inst.ins, ScopedClock({None: tick_clock.global_clock})
        )
        mi = drain_inst.ins
        waits = list(mi.sync_info.on_wait or []) if mi.sync_info else []
        if len(waits) > MAX_WAITS:
            mi.sync_info.on_wait = waits[:MAX_WAITS]
            rest = waits[MAX_WAITS:]
            while rest:
                chunk, rest = rest[:MAX_WAITS], rest[MAX_WAITS:]
                nop_inst = self.nc.sync.nop(nofuse=True)
                si = nop_inst.ins.sync_info
                if si is None:
                    nop_inst.ins.sync_info = mybir.SyncInfo(
                        on_wait=chunk, on_update=[]
                    )
                else:
                    si.on_wait = (list(si.on_wait) if si.on_wait else []) + chunk

        self.nc.all_engine_barrier()
        assert self.sems is not None
        popped = self.nc._tile_sem_poison_stack.pop()
        assert popped is self._sem_poison
        self.nc.clear_and_free_semaphores(list(self.sems.allocated().values()))
        self.nc.all_engine_barrier()


def _split_waits(nc, max_waits=MAX_WAITS):
    """This walrus build rejects any instruction carrying more than a couple of
    sync waits ("Too many sync wait commands"). Hoist excess waits onto
    same-engine NOPs inserted immediately before the instruction — engines
    dispatch in order, so blocking on the NOP first is equivalent."""
    ctr = 0
    for func in nc.m.functions:
        for bb in func.blocks:
            out = []
            for inst in bb.instructions:
                si = inst.sync_info
                waits = list(si.on_wait) if si is not None and si.on_wait else []
                if len(waits) > max_waits:
                    pre, keep = waits[:-max_waits], waits[-max_waits:]
                    while pre:
                        chunk, pre = pre[:max_waits], pre[max_waits:]
                        out.append(
                            mybir.InstNoOp(
                                name=f"I-waitsplit-{ctr}",
                                engine=inst.engine,
                                sync_info=mybir.SyncInfo(on_wait=chunk, on_update=[]),
                            )
                        )
                        ctr += 1
                    si.on_wait = keep
                out.append(inst)
            bb.instructions[:] = out


def _build_nc():
    f32 = mybir.dt.float32
    nc = bass.Bass(debug=False)
    qT = nc.declare_dram_parameter("qT", [DA, NQ], f32, isOutput=False)
    gT = nc.declare_dram_parameter("gT", [DA, B * GL_PAD], f32, isOutput=False)
    out_val = nc.declare_dram_parameter("out_val", [QT, NTILE], f32, isOutput=True)
    out_idx = nc.declare_dram_parameter(
        "out_idx", [QT, NTILE], mybir.dt.uint32, isOutput=True
    )

    with SplitDrainTileContext(nc) as tc:
        with (
            tc.tile_pool(name="weights", bufs=1) as wpool,
            tc.tile_pool(name="psum", bufs=2, space="PSUM") as ppool,
            tc.tile_pool(name="small", bufs=4) as spool,
            tc.tile_pool(name="outs", bufs=1) as opool,
        ):
            qT_sb = wpool.tile([DA, NQ], f32, tag="qT")
            gT_sb = wpool.tile([DA, B * GL_PAD], f32, tag="gT")
            nc.gpsimd.dma_start(out=qT_sb[:], in_=qT[:])
            nc.gpsimd.dma_start(out=gT_sb[:], in_=gT[:])

            val_sb = opool.tile([QT, NTILE], f32, tag="vals")
            idx_sb = opool.tile([QT, NTILE, 8], mybir.dt.uint32, tag="idxs")

            for t in range(NTILE):
                b = t // (NTILE // B)
                scores = ppool.tile([QT, GL_PAD], f32, tag="scores")
                lhsT = qT_sb[:, t * QT : (t + 1) * QT]
                for c in range(NCHUNK):
                    rhs = gT_sb[:, b * GL_PAD + c * CHUNK : b * GL_PAD + (c + 1) * CHUNK]
                    nc.tensor.matmul(
                        scores[:, c * CHUNK : (c + 1) * CHUNK],
                        lhsT,
                        rhs,
                        start=True,
                        stop=True,
                    )
                # reduce writes its staging column directly; max_index reads the
                # per-partition max through a broadcast AP and writes its own
                # 8-slot staging block (the final DMA picks out slot 0).
                nc.vector.reduce_max(
                    val_sb[:, t : t + 1], scores[:], axis=mybir.AxisListType.X
                )
                nc.vector.max_index(
                    out=idx_sb[:, t, :],
                    in_max=val_sb[:, t : t + 1].to_broadcast([QT, 8]),
                    in_values=scores[:],
                )

            nc.gpsimd.dma_start(out=out_val[:], in_=val_sb[:])
            nc.gpsimd.dma_start(out=out_idx[:], in_=idx_sb[:, :, 0])
    _split_waits(nc)
    return nc


# ------------------------------------------------------------------- kernel
def kernel(inp: np.ndarray, tar: np.ndarray):
    global _CACHED_NC
    inp = np.asarray(inp, dtype=np.float32)
    tar = np.asarray(tar, dtype=np.float32)

    # host prep: patch views + database (bit-identical to the jax reference)
    ipf = _unfold(inp).reshape(B, N, D)
    tpf = _unfold(tar).reshape(B, N, D)
    gf = _g_database(tar).reshape(B, G, D)
    gn = np.einsum("bgd,bgd->bg", gf, gf).astype(np.float32)

    # augmented query matrix, transposed: [28, 4608]
    s = (tpf + ipf).astype(np.float32)  # [B, N, 27]
    qT = np.empty((DA, NQ), dtype=np.float32)
    qT[:D, :] = s.reshape(NQ, D).T
    qT[D, :] = 1.0

    # per-core augmented candidate shards, transposed: [28, B*2048]
    in_maps = []
    for k in range(NCORES):
        lo = k * G_LOC
        hi = min(lo + G_LOC, G)
        nv = hi - lo
        gTk = np.zeros((DA, B * GL_PAD), dtype=np.float32)
        gTk[D, :] = NEG
        for b in range(B):
            gTk[:D, b * GL_PAD : b * GL_PAD + nv] = gf[b, lo:hi, :].T
            gTk[D, b * GL_PAD : b * GL_PAD + nv] = -gn[b, lo:hi]
        in_maps.append({"qT": qT, "gT": gTk})

    if _CACHED_NC is None:
        _CACHED_NC = _build_nc()
    res = run_bass_kernel_spmd(_CACHED_NC, in_maps, list(range(NCORES)))

    # merge partial argmaxes (ascending core order, strict >, so global
    # first-occurrence tie-breaking is preserved)
    best_val = np.full(NQ, -np.inf, dtype=np.float32)
    best_g = np.zeros(NQ, dtype=np.int64)
    for k in range(NCORES):
        vals = res.results[k]["out_val"].T.reshape(NQ)  # [QT, NTILE] -> [NQ]
        idxs = res.results[k]["out_idx"].T.reshape(NQ).astype(np.int64)
        upd = vals > best_val
        best_val[upd] = vals[upd]
        best_g[upd] = k * G_LOC + idxs[upd]

    index = best_g.reshape(B, N)
    sel = np.take_along_axis(gf, index[..., None], axis=1)  # [B, N, D]

    loss_p = np.abs(ipf - sel).astype(np.float32)
    loss_img = _fold(loss_p.reshape(B, N, C, PS, PS), C, H, W)
    sel_img = _fold(sel.reshape(B, N, C, PS, PS), C, H, W)
    return np.float32(loss_img.mean(dtype=np.float32)), sel_img


# revision 10
# speedup vs baseline: 1.2473x; 1.2473x over previous
"""Trainium2 Bass kernel for nn_BB_loss_80298708566608 (retrieval_knn).

Reference computation: for each of B*N query patches (3x3x3 = 27-dim), find the
nearest candidate patch among G=14460 database patches (built from `tar` at 3
scales with shifted grids), under l = 0.5*||tpf-g||^2 + 0.5*||ipf-g||^2.
Since the query-norm term is constant in g, argmin_g l == argmax_g score with
  score(q, g) = (tpf_q + ipf_q) . g - ||g||^2
which is one [4608, 28] x [28, G] matmul (augmented with a constant-1 column
against -||g||^2) fused with an argmax over the candidate axis.

Device strategy (8 NeuronCores, candidate-sharded):
  - core k owns candidates [k*1808, k*1808+1808) (last core 1804), padded to a
    local 2048 with score -1e30, laid out as 4 matmul chunks of 512.
  - every core processes all 36 query tiles of 128 queries; per tile the scores
    [128, 2048] live entirely in PSUM (4 banks, double-buffered), then DVE does
    reduce_max -> broadcast -> max_index (first-occurrence argmax semantics,
    matching jnp.argmin tie-breaking).
  - outputs per core: local max value + local argmax per query; the host merges
    the 8 partial argmaxes (ascending core order, strict >, preserving global
    first-tie-break), gathers the selected patches, and computes the L1 loss
    and the folded image.

The patch database is built on the host in numpy; it is bit-identical to the
reference's jax implementation (validated).
"""

import numpy as np

import concourse.bass as bass
import concourse.mybir as mybir
from concourse.tile import TileContext
from concourse.bass_utils import run_bass_kernel_spmd
from concourse.vector_clock import ScopedClock

# ---------------------------------------------------------------- constants
PS = 3
A_CUBIC = -0.75
B, C, H, W = 2, 3, 144, 144
N = (H // PS) * (W // PS)  # 2304 queries per batch
D = C * PS * PS  # 27
DA = D + 1  # augmented with constant-1 column
G = 14460  # database size per batch
NCORES = 8
G_LOC = 1808  # candidates per core (last core: 1804 valid)
CHUNK = 512
NCHUNK = 4
GL_PAD = CHUNK * NCHUNK  # 2048
NQ = B * N  # 4608 total queries
QT = 128  # queries per tile
NTILE = NQ // QT  # 36
NEG = -1.0e30

_CACHED_NC = None


# ------------------------------------------------------------- database build
def _cubic_w(t):
    at = np.abs(t)
    w1 = ((A_CUBIC + 2.0) * at - (A_CUBIC + 3.0)) * at * at + 1.0
    w2 = A_CUBIC * (((at - 5.0) * at + 8.0) * at - 4.0)
    return np.where(at <= 1.0, w1, np.where(at < 2.0, w2, 0.0)).astype(t.dtype)


def _resize_axis(x, out_size, axis):
    in_size = x.shape[axis]
    scale = (in_size - 1) / (out_size - 1)
    coords = np.arange(out_size, dtype=x.dtype) * np.asarray(scale, x.dtype)
    i0 = np.floor(coords)
    frac = coords - i0
    offs = np.arange(-1, 3)
    idx = np.clip(i0.astype(np.int32)[:, None] + offs[None, :], 0, in_size - 1)
    w = _cubic_w(frac[:, None] - offs[None, :].astype(x.dtype))
    xm = np.moveaxis(x, axis, -1)
    vals = xm[..., idx]
    out = np.einsum("...ot,ot->...o", vals, w)
    return np.moveaxis(out, -1, axis).astype(x.dtype)


def _bicubic(x, out_h, out_w):
    return _resize_axis(_resize_axis(x, out_h, 2), out_w, 3)


def _unfold(x, ps=PS):
    b, c, h, w = x.shape
    x = x.reshape(b, c, h // ps, ps, w // ps, ps)
    x = x.transpose(0, 2, 4, 1, 3, 5)
    return x.reshape(b, -1, c, ps, ps)


def _fold(p, c, h, w, ps=PS):
    b = p.shape[0]
    x = p.reshape(b, h // ps, w // ps, c, ps, ps)
    x = x.transpose(0, 3, 1, 4, 2, 5)
    return x.reshape(b, c, h, w)


def _g_database(tar, ps=PS):
    b, c, h, w = tar.shape
    scales = [tar, _bicubic(tar, h // 2, w // 2), _bicubic(tar, h // 4, w // 4)]
    patches = []
    for i in range(1, ps):
        for j in range(1, ps):
            for s in scales:
                sh, sw = s.shape[2], s.shape[3]
                if ps < min(sh, sw):
                    patches.append(
                        _unfold(s[:, :, i : sh - (ps - i), j : sw - (ps - j)], ps)
                    )
    for s in scales:
        patches.append(_unfold(s, ps))
    return np.concatenate(patches, axis=1)


# ------------------------------------------------------------------ bass IR
MAX_WAITS = 1


class SplitDrainTileContext(TileContext):
    """This walrus build rejects instructions carrying more than a couple of
    sync waits; split the tail-drain waits across extra SP nops (all before the
    all-engine barrier, so semantics are unchanged)."""

    def _drain_and_barrier(self, tick_clock, wait_clock):
        drain_inst = self.nc.sync.drain()
        wait_clock.add_sem_waits(
            drain_inst.ins, ScopedClock({None: tick_clock.global_clock})
        )
        mi = drain_inst.ins
        waits = list(mi.sync_info.on_wait or []) if mi.sync_info else []
        if len(waits) > MAX_WAITS:
            mi.sync_info.on_wait = waits[:MAX_WAITS]
            rest = waits[MAX_WAITS:]
            while rest:
                chunk, rest = rest[:MAX_WAITS], rest[MAX_WAITS:]
                nop_inst = self.nc.sync.nop(nofuse=True)
                si = nop_inst.ins.sync_info
                if si is None:
                    nop_inst.ins.sync_info = mybir.SyncInfo(
                        on_wait=chunk, on_update=[]
                    )
                else:
                    si.on_wait = (list(si.on_wait) if si.on_wait else []) + chunk

        self.nc.all_engine_barrier()
        assert self.sems is not None
        popped = self.nc._tile_sem_poison_stack.pop()
        assert popped is self._sem_poison
        self.nc.clear_and_free_semaphores(list(self.sems.allocated().values()))
        self.nc.all_engine_barrier()


def _split_waits(nc, max_waits=MAX_WAITS):
    """This walrus build rejects any instruction carrying more than a couple of
    sync waits ("Too many sync wait commands"). Hoist excess waits onto
    same-engine NOPs inserted immediately before the instruction — engines
    dispatch in order, so blocking on the NOP first is equivalent."""
    ctr = 0
    for func in nc.m.functions:
        for bb in func.blocks:
            out = []
            for inst in bb.instructions:
                si = inst.sync_info
                waits = list(si.on_wait) if si is not None and si.on_wait else []
                if len(waits) > max_waits:
                    pre, keep = waits[:-max_waits], waits[-max_waits:]
                    while pre:
                        chunk, pre = pre[:max_waits], pre[max_waits:]
                        out.append(
                            mybir.InstNoOp(
                                name=f"I-waitsplit-{ctr}",
                                engine=inst.engine,
                                sync_info=mybir.SyncInfo(on_wait=chunk, on_update=[]),
                            )
                        )
                        ctr += 1
                    si.on_wait = keep
                out.append(inst)
            bb.instructions[:] = out


def _build_nc():
    f32 = mybir.dt.float32
    nc = bass.Bass(debug=False)
    # Both operand tensors are stored in 4 partition bands of 32 (one per PE
    # row group): band q carries the 28 augmented-feature rows for chunk q so
    # the 4 K=28 matmuls of a tile run concurrently in 4 row groups.
    qT = nc.declare_dram_parameter("qT", [128, NQ], f32, isOutput=False)
    gT = nc.declare_dram_parameter("gT", [128, B * CHUNK], f32, isOutput=False)
    out_val = nc.declare_dram_parameter("out_val", [QT, NTILE], f32, isOutput=True)
    out_idx = nc.declare_dram_parameter(
        "out_idx", [QT, NTILE], mybir.dt.uint32, isOutput=True
    )

    with SplitDrainTileContext(nc) as tc:
        with (
            tc.tile_pool(name="weights", bufs=1) as wpool,
            tc.tile_pool(name="psum", bufs=2, space="PSUM") as ppool,
            tc.tile_pool(name="small", bufs=4) as spool,
            tc.tile_pool(name="outs", bufs=1) as opool,
        ):
            qT_sb = wpool.tile([128, NQ], f32, tag="qT")
            gT_sb = wpool.tile([128, B * CHUNK], f32, tag="gT")
            nc.gpsimd.dma_start(out=qT_sb[:], in_=qT[:])
            nc.gpsimd.dma_start(out=gT_sb[:], in_=gT[:])

            val_sb = opool.tile([QT, NTILE], f32, tag="vals")
            idx_sb = opool.tile([QT, NTILE, 8], mybir.dt.uint32, tag="idxs")

            for t in range(NTILE):
                b = t // (NTILE // B)
                scores = ppool.tile([QT, GL_PAD], f32, tag="scores")
                for c in range(NCHUNK):
                    band = 32 * c
                    nc.tensor.matmul(
                        scores[:, c * CHUNK : (c + 1) * CHUNK],
                        qT_sb[band : band + DA, t * QT : (t + 1) * QT],
                        gT_sb[band : band + DA, b * CHUNK : (b + 1) * CHUNK],
                        start=True,
                        stop=True,
                        tile_position=(band, 0),
                    )
                # reduce writes its staging column directly; max_index reads the
                # per-partition max through a broadcast AP and writes its own
                # 8-slot staging block (the final DMA picks out slot 0).
                nc.vector.reduce_max(
                    val_sb[:, t : t + 1], scores[:], axis=mybir.AxisListType.X
                )
                nc.vector.max_index(
                    out=idx_sb[:, t, :],
                    in_max=val_sb[:, t : t + 1].to_broadcast([QT, 8]),
                    in_values=scores[:],
                )

            nc.gpsimd.dma_start(out=out_val[:], in_=val_sb[:])
            nc.gpsimd.dma_start(out=out_idx[:], in_=idx_sb[:, :, 0])
    _split_waits(nc)
    return nc


# ------------------------------------------------------------------- kernel
def _prepare(inp: np.ndarray, tar: np.ndarray):
    """Host prep: patch views, database (bit-identical to the jax reference),
    and the per-core banded input tensors."""
    ipf = _unfold(inp).reshape(B, N, D)
    tpf = _unfold(tar).reshape(B, N, D)
    gf = _g_database(tar).reshape(B, G, D)
    gn = np.einsum("bgd,bgd->bg", gf, gf).astype(np.float32)

    # augmented query matrix, transposed and replicated into the 4 PE
    # row-group bands: [128, 4608]
    s = (tpf + ipf).astype(np.float32)  # [B, N, 27]
    qT = np.zeros((128, NQ), dtype=np.float32)
    for q in range(NCHUNK):
        qT[32 * q : 32 * q + D, :] = s.reshape(NQ, D).T
        qT[32 * q + D, :] = 1.0

    # per-core augmented candidate shards: band q carries chunk q: [128, B*512]
    in_maps = []
    for k in range(NCORES):
        lo = k * G_LOC
        hi = min(lo + G_LOC, G)
        nv = hi - lo
        gTk = np.zeros((128, B * CHUNK), dtype=np.float32)
        ga = np.zeros((B, DA, GL_PAD), dtype=np.float32)
        ga[:, D, :] = NEG
        for b in range(B):
            ga[b, :D, :nv] = gf[b, lo:hi, :].T
            ga[b, D, :nv] = -gn[b, lo:hi]
        for q in range(NCHUNK):
            for b in range(B):
                gTk[32 * q : 32 * q + DA, b * CHUNK : (b + 1) * CHUNK] = ga[
                    b, :, q * CHUNK : (q + 1) * CHUNK
                ]
        in_maps.append({"qT": qT, "gT": gTk})
    return in_maps, ipf, gf


def kernel(inp: np.ndarray, tar: np.ndarray):
    global _CACHED_NC
    inp = np.asarray(inp, dtype=np.float32)
    tar = np.asarray(tar, dtype=np.float32)

    in_maps, ipf, gf = _prepare(inp, tar)

    if _CACHED_NC is None:
        _CACHED_NC = _build_nc()
    res = run_bass_kernel_spmd(_CACHED_NC, in_maps, list(range(NCORES)))

    # merge partial argmaxes (ascending core order, strict >, so global
    # first-occurrence tie-breaking is preserved)
    best_val = np.full(NQ, -np.inf, dtype=np.float32)
    best_g = np.zeros(NQ, dtype=np.int64)
    for k in range(NCORES):
        vals = res.results[k]["out_val"].T.reshape(NQ)  # [QT, NTILE] -> [NQ]
        idxs = res.results[k]["out_idx"].T.reshape(NQ).astype(np.int64)
        upd = vals > best_val
        best_val[upd] = vals[upd]
        best_g[upd] = k * G_LOC + idxs[upd]

    index = best_g.reshape(B, N)
    sel = np.take_along_axis(gf, index[..., None], axis=1)  # [B, N, D]

    loss_p = np.abs(ipf - sel).astype(np.float32)
    loss_img = _fold(loss_p.reshape(B, N, C, PS, PS), C, H, W)
    sel_img = _fold(sel.reshape(B, N, C, PS, PS), C, H, W)
    return np.float32(loss_img.mean(dtype=np.float32)), sel_img


# revision 22
# speedup vs baseline: 1.6389x; 1.3139x over previous
"""Trainium2 Bass kernel for nn_BB_loss_80298708566608 (retrieval_knn).

Reference computation: for each of B*N query patches (3x3x3 = 27-dim), find the
nearest candidate patch among G=14460 database patches (built from `tar` at 3
scales with shifted grids), under l = 0.5*||tpf-g||^2 + 0.5*||ipf-g||^2.
Since the query-norm term is constant in g, argmin_g l == argmax_g score with
  score(q, g) = (tpf_q + ipf_q) . g - ||g||^2
which is one [4608, 28] x [28, G] matmul (augmented with a constant-1 column
against -||g||^2) fused with an argmax over the candidate axis.

Device strategy (8 NeuronCores, candidate-sharded):
  - core k owns candidates [k*1808, k*1808+1808) (last core: 1804 real + 4
    filler rows whose augmented column is -1e30 so they never win), split into
    4 chunks of 452.
  - every core processes all 36 query tiles of 128 queries. Per tile: the 4
    K=28 fp32 matmuls run CONCURRENTLY in the 4 PE row groups (operands stored
    in 32-partition bands, tile_position=(32q,0)), each writing one PSUM bank;
    ACT packs the 4x452 used columns into one contiguous SBUF row; DVE then
    does reduce_max + max_index over [128,1808] (max_index returns
    first-occurrence indices = jnp.argmin tie semantics).
  - outputs per core: local max value + local argmax per query; the host merges
    the 8 partial argmaxes (ascending core order, strict >, preserving global
    first-tie-break), gathers the selected patches, and computes the L1 loss
    and the folded image.

The patch database is built on the host in numpy; it is bit-identical to the
reference's jax implementation (validated).
"""

import numpy as np

import concourse.bass as bass
import concourse.mybir as mybir
from concourse.tile import TileContext
from concourse.bass_utils import run_bass_kernel_spmd
from concourse.vector_clock import ScopedClock

# ---------------------------------------------------------------- constants
PS = 3
A_CUBIC = -0.75
B, C, H, W = 2, 3, 144, 144
N = (H // PS) * (W // PS)  # 2304 queries per batch
D = C * PS * PS  # 27
DA = D + 1  # augmented with constant-1 column
G = 14460  # database size per batch
NCORES = 8
G_LOC = 1808  # candidates per core (last core: 1804 valid)
NCHUNK = 4
CHUNK = G_LOC // NCHUNK  # 452 — matmul free dim per PSUM bank (bank holds 512)
BANK = 512
NQ = B * N  # 4608 total queries
QT = 128  # queries per tile
NTILE = NQ // QT  # 36
NEG = -1.0e30

_CACHED_NC = None


# ------------------------------------------------------------- database build
def _cubic_w(t):
    at = np.abs(t)
    w1 = ((A_CUBIC + 2.0) * at - (A_CUBIC + 3.0)) * at * at + 1.0
    w2 = A_CUBIC * (((at - 5.0) * at + 8.0) * at - 4.0)
    return np.where(at <= 1.0, w1, np.where(at < 2.0, w2, 0.0)).astype(t.dtype)


def _resize_axis(x, out_size, axis):
    in_size = x.shape[axis]
    scale = (in_size - 1) / (out_size - 1)
    coords = np.arange(out_size, dtype=x.dtype) * np.asarray(scale, x.dtype)
    i0 = np.floor(coords)
    frac = coords - i0
    offs = np.arange(-1, 3)
    idx = np.clip(i0.astype(np.int32)[:, None] + offs[None, :], 0, in_size - 1)
    w = _cubic_w(frac[:, None] - offs[None, :].astype(x.dtype))
    xm = np.moveaxis(x, axis, -1)
    vals = xm[..., idx]
    out = np.einsum("...ot,ot->...o", vals, w)
    return np.moveaxis(out, -1, axis).astype(x.dtype)


def _bicubic(x, out_h, out_w):
    return _resize_axis(_resize_axis(x, out_h, 2), out_w, 3)


def _unfold(x, ps=PS):
    b, c, h, w = x.shape
    x = x.reshape(b, c, h // ps, ps, w // ps, ps)
    x = x.transpose(0, 2, 4, 1, 3, 5)
    return x.reshape(b, -1, c, ps, ps)


def _fold(p, c, h, w, ps=PS):
    b = p.shape[0]
    x = p.reshape(b, h // ps, w // ps, c, ps, ps)
    x = x.transpose(0, 3, 1, 4, 2, 5)
    return x.reshape(b, c, h, w)


def _g_database(tar, ps=PS):
    b, c, h, w = tar.shape
    scales = [tar, _bicubic(tar, h // 2, w // 2), _bicubic(tar, h // 4, w // 4)]
    patches = []
    for i in range(1, ps):
        for j in range(1, ps):
            for s in scales:
                sh, sw = s.shape[2], s.shape[3]
                if ps < min(sh, sw):
                    patches.append(
                        _unfold(s[:, :, i : sh - (ps - i), j : sw - (ps - j)], ps)
                    )
    for s in scales:
        patches.append(_unfold(s, ps))
    return np.concatenate(patches, axis=1)


# ------------------------------------------------------------------ bass IR
MAX_WAITS = 1


class SplitDrainTileContext(TileContext):
    """This walrus build rejects instructions carrying more than a couple of
    sync waits; split the tail-drain waits across extra SP nops (all before the
    all-engine barrier, so semantics are unchanged)."""

    def _drain_and_barrier(self, tick_clock, wait_clock):
        drain_inst = self.nc.sync.drain()
        wait_clock.add_sem_waits(
            drain_inst.ins, ScopedClock({None: tick_clock.global_clock})
        )
        mi = drain_inst.ins
        waits = list(mi.sync_info.on_wait or []) if mi.sync_info else []
        if len(waits) > MAX_WAITS:
            mi.sync_info.on_wait = waits[:MAX_WAITS]
            rest = waits[MAX_WAITS:]
            while rest:
                chunk, rest = rest[:MAX_WAITS], rest[MAX_WAITS:]
                nop_inst = self.nc.sync.nop(nofuse=True)
                si = nop_inst.ins.sync_info
                if si is None:
                    nop_inst.ins.sync_info = mybir.SyncInfo(
                        on_wait=chunk, on_update=[]
                    )
                else:
                    si.on_wait = (list(si.on_wait) if si.on_wait else []) + chunk

        self.nc.all_engine_barrier()
        assert self.sems is not None
        popped = self.nc._tile_sem_poison_stack.pop()
        assert popped is self._sem_poison
        self.nc.clear_and_free_semaphores(list(self.sems.allocated().values()))
        self.nc.all_engine_barrier()


def _split_waits(nc, max_waits=MAX_WAITS):
    """This walrus build rejects any instruction carrying more than a couple of
    sync waits ("Too many sync wait commands"). Hoist excess waits onto
    same-engine NOPs inserted immediately before the instruction — engines
    dispatch in order, so blocking on the NOP first is equivalent."""
    ctr = 0
    for func in nc.m.functions:
        for bb in func.blocks:
            out = []
            for inst in bb.instructions:
                si = inst.sync_info
                waits = list(si.on_wait) if si is not None and si.on_wait else []
                if len(waits) > max_waits:
                    pre, keep = waits[:-max_waits], waits[-max_waits:]
                    while pre:
                        chunk, pre = pre[:max_waits], pre[max_waits:]
                        out.append(
                            mybir.InstNoOp(
                                name=f"I-waitsplit-{ctr}",
                                engine=inst.engine,
                                sync_info=mybir.SyncInfo(on_wait=chunk, on_update=[]),
                            )
                        )
                        ctr += 1
                    si.on_wait = keep
                out.append(inst)
            bb.instructions[:] = out


def _build_nc():
    f32 = mybir.dt.float32
    nc = bass.Bass(debug=False)
    # Both operand tensors are stored in 4 partition bands of 32 (one per PE
    # row group): band q carries the 28 augmented-feature rows for chunk q so
    # the 4 K=28 matmuls of a tile run concurrently in 4 row groups.
    qT = nc.declare_dram_parameter("qT", [128, NQ], f32, isOutput=False)
    gT = nc.declare_dram_parameter("gT", [128, B * CHUNK], f32, isOutput=False)
    out_val = nc.declare_dram_parameter("out_val", [QT, NTILE], f32, isOutput=True)
    # full 8-slot max_index staging is DMA'd contiguously; host picks slot 0
    out_idx = nc.declare_dram_parameter(
        "out_idx", [QT, NTILE, 8], mybir.dt.uint32, isOutput=True
    )

    with SplitDrainTileContext(nc) as tc:
        with (
            tc.tile_pool(name="weights", bufs=1) as wpool,
            tc.tile_pool(name="psum", bufs=2, space="PSUM") as ppool,
            tc.tile_pool(name="small", bufs=4) as spool,
            tc.tile_pool(name="scratch", bufs=2) as cpool,
            tc.tile_pool(name="outs", bufs=1) as opool,
        ):
            gT_sb = wpool.tile([128, B * CHUNK], f32, tag="gT")
            nc.gpsimd.dma_start(out=gT_sb[:], in_=gT[:])
            # chunk the query DMA into separate tiles so the first matmuls
            # don't wait for the full 2.4MB transfer
            QCHUNK = 6  # tiles per query-DMA piece
            qT_sbs = []
            for j in range(NTILE // QCHUNK):
                qp = wpool.tile([128, QCHUNK * QT], f32, tag=f"qT{j}")
                nc.gpsimd.dma_start(
                    out=qp[:], in_=qT[:, j * QCHUNK * QT : (j + 1) * QCHUNK * QT]
                )
                qT_sbs.append(qp)

            val_sb = opool.tile([QT, NTILE], f32, tag="vals")
            idx_sb = opool.tile([QT, NTILE, 8], mybir.dt.uint32, tag="idxs")

            for t in range(NTILE):
                b = t // (NTILE // B)
                # one PSUM bank per 452-wide chunk (cols 452..511 unused)
                scores = ppool.tile([QT, NCHUNK, BANK], f32, tag="scores")
                qp = qT_sbs[t // QCHUNK]
                qoff = (t % QCHUNK) * QT
                for c in range(NCHUNK):
                    band = 32 * c
                    nc.tensor.matmul(
                        scores[:, c, :CHUNK],
                        qp[band : band + DA, qoff : qoff + QT],
                        gT_sb[band : band + DA, b * CHUNK : (b + 1) * CHUNK],
                        start=True,
                        stop=True,
                        tile_position=(band, 0),
                    )
                # ACT packs the used columns of the 4 banks into one contiguous
                # SBUF buffer; both DVE passes then scan 1808 (not 2048) from
                # SBUF, and the found index IS the core-local candidate id.
                sc = cpool.tile([QT, G_LOC], f32, tag="sc")
                for c in range(NCHUNK):
                    nc.scalar.copy(
                        sc[:, c * CHUNK : (c + 1) * CHUNK], scores[:, c, :CHUNK]
                    )
                # reduce writes its staging column directly; max_index reads the
                # per-partition max through a broadcast AP and writes its own
                # 8-slot staging block (the final DMA picks out slot 0).
                nc.vector.reduce_max(
                    val_sb[:, t : t + 1], sc[:], axis=mybir.AxisListType.X
                )
                nc.vector.max_index(
                    out=idx_sb[:, t, :],
                    in_max=val_sb[:, t : t + 1].to_broadcast([QT, 8]),
                    in_values=sc[:],
                )

            nc.gpsimd.dma_start(out=out_val[:], in_=val_sb[:])
            nc.gpsimd.dma_start(out=out_idx[:], in_=idx_sb[:])
    _split_waits(nc)
    return nc


# ------------------------------------------------------------------- kernel
def _prepare(inp: np.ndarray, tar: np.ndarray):
    """Host prep: patch views, database (bit-identical to the jax reference),
    and the per-core banded input tensors."""
    ipf = _unfold(inp).reshape(B, N, D)
    tpf = _unfold(tar).reshape(B, N, D)
    gf = _g_database(tar).reshape(B, G, D)
    gn = np.einsum("bgd,bgd->bg", gf, gf).astype(np.float32)

    # augmented query matrix, transposed and replicated into the 4 PE
    # row-group bands: [128, 4608]
    s = (tpf + ipf).astype(np.float32)  # [B, N, 27]
    qT = np.zeros((128, NQ), dtype=np.float32)
    for q in range(NCHUNK):
        qT[32 * q : 32 * q + D, :] = s.reshape(NQ, D).T
        qT[32 * q + D, :] = 1.0

    # per-core augmented candidate shards: band q carries chunk q: [128, B*512]
    in_maps = []
    for k in range(NCORES):
        lo = k * G_LOC
        hi = min(lo + G_LOC, G)
        nv = hi - lo
        gTk = np.zeros((128, B * CHUNK), dtype=np.float32)
        ga = np.zeros((B, DA, G_LOC), dtype=np.float32)
        ga[:, D, :] = NEG
        for b in range(B):
            ga[b, :D, :nv] = gf[b, lo:hi, :].T
            ga[b, D, :nv] = -gn[b, lo:hi]
        for q in range(NCHUNK):
            for b in range(B):
                gTk[32 * q : 32 * q + DA, b * CHUNK : (b + 1) * CHUNK] = ga[
                    b, :, q * CHUNK : (q + 1) * CHUNK
                ]
        in_maps.append({"qT": qT, "gT": gTk})
    return in_maps, ipf, gf


def kernel(inp: np.ndarray, tar: np.ndarray):
    global _CACHED_NC
    inp = np.asarray(inp, dtype=np.float32)
    tar = np.asarray(tar, dtype=np.float32)

    in_maps, ipf, gf = _prepare(inp, tar)

    if _CACHED_NC is None:
        _CACHED_NC = _build_nc()
    res = run_bass_kernel_spmd(_CACHED_NC, in_maps, list(range(NCORES)))

    # merge partial argmaxes (ascending core order, strict >, so global
    # first-occurrence tie-breaking is preserved)
    best_val = np.full(NQ, -np.inf, dtype=np.float32)
    best_g = np.zeros(NQ, dtype=np.int64)
    for k in range(NCORES):
        vals = res.results[k]["out_val"].T.reshape(NQ)  # [QT, NTILE] -> [NQ]
        idxs = (
            res.results[k]["out_idx"][:, :, 0].T.reshape(NQ).astype(np.int64)
        )
        upd = vals > best_val
        best_val[upd] = vals[upd]
        best_g[upd] = k * G_LOC + idxs[upd]

    index = best_g.reshape(B, N)
    sel = np.take_along_axis(gf, index[..., None], axis=1)  # [B, N, D]

    loss_p = np.abs(ipf - sel).astype(np.float32)
    loss_img = _fold(loss_p.reshape(B, N, C, PS, PS), C, H, W)
    sel_img = _fold(sel.reshape(B, N, C, PS, PS), C, H, W)
    return np.float32(loss_img.mean(dtype=np.float32)), sel_img


# revision 23
# speedup vs baseline: 1.6417x; 1.0018x over previous
"""Trainium2 Bass kernel for nn_BB_loss_80298708566608 (retrieval_knn).

Reference computation: for each of B*N query patches (3x3x3 = 27-dim), find the
nearest candidate patch among G=14460 database patches (built from `tar` at 3
scales with shifted grids), under l = 0.5*||tpf-g||^2 + 0.5*||ipf-g||^2.
Since the query-norm term is constant in g, argmin_g l == argmax_g score with
  score(q, g) = (tpf_q + ipf_q) . g - ||g||^2
which is one [4608, 28] x [28, G] matmul (augmented with a constant-1 column
against -||g||^2) fused with an argmax over the candidate axis.

Device strategy (8 NeuronCores, candidate-sharded):
  - core k owns candidates [k*1808, k*1808+1808) (last core: 1804 real + 4
    filler rows whose augmented column is -1e30 so they never win), split into
    4 chunks of 452.
  - every core processes all 36 query tiles of 128 queries. Per tile: the 4
    K=28 fp32 matmuls run CONCURRENTLY in the 4 PE row groups (operands stored
    in 32-partition bands, tile_position=(32q,0)), each writing one PSUM bank;
    ACT packs the 4x452 used columns into one contiguous SBUF row; DVE then
    does reduce_max + max_index over [128,1808] (max_index returns
    first-occurrence indices = jnp.argmin tie semantics).
  - outputs per core: local max value + local argmax per query; the host merges
    the 8 partial argmaxes (ascending core order, strict >, preserving global
    first-tie-break), gathers the selected patches, and computes the L1 loss
    and the folded image.

The patch database is built on the host in numpy; it is bit-identical to the
reference's jax implementation (validated).
"""

import numpy as np

import concourse.bass as bass
import concourse.mybir as mybir
from concourse.tile import TileContext
from concourse.bass_utils import run_bass_kernel_spmd
from concourse.vector_clock import ScopedClock

# ---------------------------------------------------------------- constants
PS = 3
A_CUBIC = -0.75
B, C, H, W = 2, 3, 144, 144
N = (H // PS) * (W // PS)  # 2304 queries per batch
D = C * PS * PS  # 27
DA = D + 1  # augmented with constant-1 column
G = 14460  # database size per batch
NCORES = 8
G_LOC = 1808  # candidates per core (last core: 1804 valid)
NCHUNK = 4
CHUNK = G_LOC // NCHUNK  # 452 — matmul free dim per PSUM bank (bank holds 512)
BANK = 512
NQ = B * N  # 4608 total queries
QT = 128  # queries per tile
NTILE = NQ // QT  # 36
NEG = -1.0e30

_CACHED_NC = None


# ------------------------------------------------------------- database build
def _cubic_w(t):
    at = np.abs(t)
    w1 = ((A_CUBIC + 2.0) * at - (A_CUBIC + 3.0)) * at * at + 1.0
    w2 = A_CUBIC * (((at - 5.0) * at + 8.0) * at - 4.0)
    return np.where(at <= 1.0, w1, np.where(at < 2.0, w2, 0.0)).astype(t.dtype)


def _resize_axis(x, out_size, axis):
    in_size = x.shape[axis]
    scale = (in_size - 1) / (out_size - 1)
    coords = np.arange(out_size, dtype=x.dtype) * np.asarray(scale, x.dtype)
    i0 = np.floor(coords)
    frac = coords - i0
    offs = np.arange(-1, 3)
    idx = np.clip(i0.astype(np.int32)[:, None] + offs[None, :], 0, in_size - 1)
    w = _cubic_w(frac[:, None] - offs[None, :].astype(x.dtype))
    xm = np.moveaxis(x, axis, -1)
    vals = xm[..., idx]
    out = np.einsum("...ot,ot->...o", vals, w)
    return np.moveaxis(out, -1, axis).astype(x.dtype)


def _bicubic(x, out_h, out_w):
    return _resize_axis(_resize_axis(x, out_h, 2), out_w, 3)


def _unfold(x, ps=PS):
    b, c, h, w = x.shape
    x = x.reshape(b, c, h // ps, ps, w // ps, ps)
    x = x.transpose(0, 2, 4, 1, 3, 5)
    return x.reshape(b, -1, c, ps, ps)


def _fold(p, c, h, w, ps=PS):
    b = p.shape[0]
    x = p.reshape(b, h // ps, w // ps, c, ps, ps)
    x = x.transpose(0, 3, 1, 4, 2, 5)
    return x.reshape(b, c, h, w)


def _g_database(tar, ps=PS):
    b, c, h, w = tar.shape
    scales = [tar, _bicubic(tar, h // 2, w // 2), _bicubic(tar, h // 4, w // 4)]
    patches = []
    for i in range(1, ps):
        for j in range(1, ps):
            for s in scales:
                sh, sw = s.shape[2], s.shape[3]
                if ps < min(sh, sw):
                    patches.append(
                        _unfold(s[:, :, i : sh - (ps - i), j : sw - (ps - j)], ps)
                    )
    for s in scales:
        patches.append(_unfold(s, ps))
    return np.concatenate(patches, axis=1)


# ------------------------------------------------------------------ bass IR
MAX_WAITS = 1


class SplitDrainTileContext(TileContext):
    """This walrus build rejects instructions carrying more than a couple of
    sync waits; split the tail-drain waits across extra SP nops (all before the
    all-engine barrier, so semantics are unchanged)."""

    def _drain_and_barrier(self, tick_clock, wait_clock):
        drain_inst = self.nc.sync.drain()
        wait_clock.add_sem_waits(
            drain_inst.ins, ScopedClock({None: tick_clock.global_clock})
        )
        mi = drain_inst.ins
        waits = list(mi.sync_info.on_wait or []) if mi.sync_info else []
        if len(waits) > MAX_WAITS:
            mi.sync_info.on_wait = waits[:MAX_WAITS]
            rest = waits[MAX_WAITS:]
            while rest:
                chunk, rest = rest[:MAX_WAITS], rest[MAX_WAITS:]
                nop_inst = self.nc.sync.nop(nofuse=True)
                si = nop_inst.ins.sync_info
                if si is None:
                    nop_inst.ins.sync_info = mybir.SyncInfo(
                        on_wait=chunk, on_update=[]
                    )
                else:
                    si.on_wait = (list(si.on_wait) if si.on_wait else []) + chunk

        self.nc.all_engine_barrier()
        assert self.sems is not None
        popped = self.nc._tile_sem_poison_stack.pop()
        assert popped is self._sem_poison
        self.nc.clear_and_free_semaphores(list(self.sems.allocated().values()))
        self.nc.all_engine_barrier()


def _split_waits(nc, max_waits=MAX_WAITS):
    """This walrus build rejects any instruction carrying more than a couple of
    sync waits ("Too many sync wait commands"). Hoist excess waits onto
    same-engine NOPs inserted immediately before the instruction — engines
    dispatch in order, so blocking on the NOP first is equivalent."""
    ctr = 0
    for func in nc.m.functions:
        for bb in func.blocks:
            out = []
            for inst in bb.instructions:
                si = inst.sync_info
                waits = list(si.on_wait) if si is not None and si.on_wait else []
                if len(waits) > max_waits:
                    pre, keep = waits[:-max_waits], waits[-max_waits:]
                    while pre:
                        chunk, pre = pre[:max_waits], pre[max_waits:]
                        out.append(
                            mybir.InstNoOp(
                                name=f"I-waitsplit-{ctr}",
                                engine=inst.engine,
                                sync_info=mybir.SyncInfo(on_wait=chunk, on_update=[]),
                            )
                        )
                        ctr += 1
                    si.on_wait = keep
                out.append(inst)
            bb.instructions[:] = out


def _build_nc():
    f32 = mybir.dt.float32
    nc = bass.Bass(debug=False)
    # Both operand tensors are stored in 4 partition bands of 32 (one per PE
    # row group): band q carries the 28 augmented-feature rows for chunk q so
    # the 4 K=28 matmuls of a tile run concurrently in 4 row groups.
    qT = nc.declare_dram_parameter("qT", [128, NQ], f32, isOutput=False)
    gT = nc.declare_dram_parameter("gT", [128, B * CHUNK], f32, isOutput=False)
    out_val = nc.declare_dram_parameter("out_val", [QT, NTILE], f32, isOutput=True)
    # full 8-slot max_index staging is DMA'd contiguously; host picks slot 0
    out_idx = nc.declare_dram_parameter(
        "out_idx", [QT, NTILE, 8], mybir.dt.uint32, isOutput=True
    )

    with SplitDrainTileContext(nc) as tc:
        with (
            tc.tile_pool(name="weights", bufs=1) as wpool,
            tc.tile_pool(name="psum", bufs=2, space="PSUM") as ppool,
            tc.tile_pool(name="small", bufs=4) as spool,
            tc.tile_pool(name="scratch", bufs=3) as cpool,
            tc.tile_pool(name="outs", bufs=1) as opool,
        ):
            gT_sb = wpool.tile([128, B * CHUNK], f32, tag="gT")
            nc.gpsimd.dma_start(out=gT_sb[:], in_=gT[:])
            # chunk the query DMA into separate tiles so the first matmuls
            # don't wait for the full 2.4MB transfer
            QCHUNK = 6  # tiles per query-DMA piece
            qT_sbs = []
            for j in range(NTILE // QCHUNK):
                qp = wpool.tile([128, QCHUNK * QT], f32, tag=f"qT{j}")
                nc.gpsimd.dma_start(
                    out=qp[:], in_=qT[:, j * QCHUNK * QT : (j + 1) * QCHUNK * QT]
                )
                qT_sbs.append(qp)

            val_sb = opool.tile([QT, NTILE], f32, tag="vals")
            idx_sb = opool.tile([QT, NTILE, 8], mybir.dt.uint32, tag="idxs")

            for t in range(NTILE):
                b = t // (NTILE // B)
                # one PSUM bank per 452-wide chunk (cols 452..511 unused)
                scores = ppool.tile([QT, NCHUNK, BANK], f32, tag="scores")
                qp = qT_sbs[t // QCHUNK]
                qoff = (t % QCHUNK) * QT
                for c in range(NCHUNK):
                    band = 32 * c
                    nc.tensor.matmul(
                        scores[:, c, :CHUNK],
                        qp[band : band + DA, qoff : qoff + QT],
                        gT_sb[band : band + DA, b * CHUNK : (b + 1) * CHUNK],
                        start=True,
                        stop=True,
                        tile_position=(band, 0),
                    )
                # ACT packs the used columns of the 4 banks into one contiguous
                # SBUF buffer; both DVE passes then scan 1808 (not 2048) from
                # SBUF, and the found index IS the core-local candidate id.
                sc = cpool.tile([QT, G_LOC], f32, tag="sc")
                for c in range(NCHUNK):
                    nc.scalar.copy(
                        sc[:, c * CHUNK : (c + 1) * CHUNK], scores[:, c, :CHUNK]
                    )
                # reduce writes its staging column directly; max_index reads the
                # per-partition max through a broadcast AP and writes its own
                # 8-slot staging block (the final DMA picks out slot 0).
                nc.vector.reduce_max(
                    val_sb[:, t : t + 1], sc[:], axis=mybir.AxisListType.X
                )
                nc.vector.max_index(
                    out=idx_sb[:, t, :],
                    in_max=val_sb[:, t : t + 1].to_broadcast([QT, 8]),
                    in_values=sc[:],
                )

            nc.gpsimd.dma_start(out=out_val[:], in_=val_sb[:])
            nc.gpsimd.dma_start(out=out_idx[:], in_=idx_sb[:])
    _split_waits(nc)
    return nc


# ------------------------------------------------------------------- kernel
def _prepare(inp: np.ndarray, tar: np.ndarray):
    """Host prep: patch views, database (bit-identical to the jax reference),
    and the per-core banded input tensors."""
    ipf = _unfold(inp).reshape(B, N, D)
    tpf = _unfold(tar).reshape(B, N, D)
    gf = _g_database(tar).reshape(B, G, D)
    gn = np.einsum("bgd,bgd->bg", gf, gf).astype(np.float32)

    # augmented query matrix, transposed and replicated into the 4 PE
    # row-group bands: [128, 4608]
    s = (tpf + ipf).astype(np.float32)  # [B, N, 27]
    qT = np.zeros((128, NQ), dtype=np.float32)
    for q in range(NCHUNK):
        qT[32 * q : 32 * q + D, :] = s.reshape(NQ, D).T
        qT[32 * q + D, :] = 1.0

    # per-core augmented candidate shards: band q carries chunk q: [128, B*512]
    in_maps = []
    for k in range(NCORES):
        lo = k * G_LOC
        hi = min(lo + G_LOC, G)
        nv = hi - lo
        gTk = np.zeros((128, B * CHUNK), dtype=np.float32)
        ga = np.zeros((B, DA, G_LOC), dtype=np.float32)
        ga[:, D, :] = NEG
        for b in range(B):
            ga[b, :D, :nv] = gf[b, lo:hi, :].T
            ga[b, D, :nv] = -gn[b, lo:hi]
        for q in range(NCHUNK):
            for b in range(B):
                gTk[32 * q : 32 * q + DA, b * CHUNK : (b + 1) * CHUNK] = ga[
                    b, :, q * CHUNK : (q + 1) * CHUNK
                ]
        in_maps.append({"qT": qT, "gT": gTk})
    return in_maps, ipf, gf


def kernel(inp: np.ndarray, tar: np.ndarray):
    global _CACHED_NC
    inp = np.asarray(inp, dtype=np.float32)
    tar = np.asarray(tar, dtype=np.float32)

    in_maps, ipf, gf = _prepare(inp, tar)

    if _CACHED_NC is None:
        _CACHED_NC = _build_nc()
    res = run_bass_kernel_spmd(_CACHED_NC, in_maps, list(range(NCORES)))

    # merge partial argmaxes (ascending core order, strict >, so global
    # first-occurrence tie-breaking is preserved)
    best_val = np.full(NQ, -np.inf, dtype=np.float32)
    best_g = np.zeros(NQ, dtype=np.int64)
    for k in range(NCORES):
        vals = res.results[k]["out_val"].T.reshape(NQ)  # [QT, NTILE] -> [NQ]
        idxs = (
            res.results[k]["out_idx"][:, :, 0].T.reshape(NQ).astype(np.int64)
        )
        upd = vals > best_val
        best_val[upd] = vals[upd]
        best_g[upd] = k * G_LOC + idxs[upd]

    index = best_g.reshape(B, N)
    sel = np.take_along_axis(gf, index[..., None], axis=1)  # [B, N, D]

    loss_p = np.abs(ipf - sel).astype(np.float32)
    loss_img = _fold(loss_p.reshape(B, N, C, PS, PS), C, H, W)
    sel_img = _fold(sel.reshape(B, N, C, PS, PS), C, H, W)
    return np.float32(loss_img.mean(dtype=np.float32)), sel_img



# revision 24
# speedup vs baseline: 1.6577x; 1.0097x over previous
"""Trainium2 Bass kernel for nn_BB_loss_80298708566608 (retrieval_knn).

Reference computation: for each of B*N query patches (3x3x3 = 27-dim), find the
nearest candidate patch among G=14460 database patches (built from `tar` at 3
scales with shifted grids), under l = 0.5*||tpf-g||^2 + 0.5*||ipf-g||^2.
Since the query-norm term is constant in g, argmin_g l == argmax_g score with
  score(q, g) = (tpf_q + ipf_q) . g - ||g||^2
which is one [4608, 28] x [28, G] matmul (augmented with a constant-1 column
against -||g||^2) fused with an argmax over the candidate axis.

Device strategy (8 NeuronCores, candidate-sharded):
  - core k owns candidates [k*1808, k*1808+1808) (last core: 1804 real + 4
    filler rows whose augmented column is -1e30 so they never win), split into
    4 chunks of 452.
  - every core processes all 36 query tiles of 128 queries. Per tile: the 4
    K=28 fp32 matmuls run CONCURRENTLY in the 4 PE row groups (operands stored
    in 32-partition bands, tile_position=(32q,0)), each writing one PSUM bank;
    ACT packs the 4x452 used columns into one contiguous SBUF row; DVE then
    does reduce_max + max_index over [128,1808] (max_index returns
    first-occurrence indices = jnp.argmin tie semantics).
  - outputs per core: local max value + local argmax per query; the host merges
    the 8 partial argmaxes (ascending core order, strict >, preserving global
    first-tie-break), gathers the selected patches, and computes the L1 loss
    and the folded image.

The patch database is built on the host in numpy; it is bit-identical to the
reference's jax implementation (validated).
"""

import numpy as np

import concourse.bass as bass
import concourse.mybir as mybir
from concourse.tile import TileContext
from concourse.bass_utils import run_bass_kernel_spmd
from concourse.vector_clock import ScopedClock

# ---------------------------------------------------------------- constants
PS = 3
A_CUBIC = -0.75
B, C, H, W = 2, 3, 144, 144
N = (H // PS) * (W // PS)  # 2304 queries per batch
D = C * PS * PS  # 27
DA = D + 1  # augmented with constant-1 column
G = 14460  # database size per batch
NCORES = 8
G_LOC = 1808  # candidates per core (last core: 1804 valid)
NCHUNK = 4
CHUNK = G_LOC // NCHUNK  # 452 — matmul free dim per PSUM bank (bank holds 512)
BANK = 512
NQ = B * N  # 4608 total queries
QT = 128  # queries per tile
NTILE = NQ // QT  # 36
NEG = -1.0e30

_CACHED_NC = None


# ------------------------------------------------------------- database build
def _cubic_w(t):
    at = np.abs(t)
    w1 = ((A_CUBIC + 2.0) * at - (A_CUBIC + 3.0)) * at * at + 1.0
    w2 = A_CUBIC * (((at - 5.0) * at + 8.0) * at - 4.0)
    return np.where(at <= 1.0, w1, np.where(at < 2.0, w2, 0.0)).astype(t.dtype)


def _resize_axis(x, out_size, axis):
    in_size = x.shape[axis]
    scale = (in_size - 1) / (out_size - 1)
    coords = np.arange(out_size, dtype=x.dtype) * np.asarray(scale, x.dtype)
    i0 = np.floor(coords)
    frac = coords - i0
    offs = np.arange(-1, 3)
    idx = np.clip(i0.astype(np.int32)[:, None] + offs[None, :], 0, in_size - 1)
    w = _cubic_w(frac[:, None] - offs[None, :].astype(x.dtype))
    xm = np.moveaxis(x, axis, -1)
    vals = xm[..., idx]
    out = np.einsum("...ot,ot->...o", vals, w)
    return np.moveaxis(out, -1, axis).astype(x.dtype)


def _bicubic(x, out_h, out_w):
    return _resize_axis(_resize_axis(x, out_h, 2), out_w, 3)


def _unfold(x, ps=PS):
    b, c, h, w = x.shape
    x = x.reshape(b, c, h // ps, ps, w // ps, ps)
    x = x.transpose(0, 2, 4, 1, 3, 5)
    return x.reshape(b, -1, c, ps, ps)


def _fold(p, c, h, w, ps=PS):
    b = p.shape[0]
    x = p.reshape(b, h // ps, w // ps, c, ps, ps)
    x = x.transpose(0, 3, 1, 4, 2, 5)
    return x.reshape(b, c, h, w)


def _g_database(tar, ps=PS):
    b, c, h, w = tar.shape
    scales = [tar, _bicubic(tar, h // 2, w // 2), _bicubic(tar, h // 4, w // 4)]
    patches = []
    for i in range(1, ps):
        for j in range(1, ps):
            for s in scales:
                sh, sw = s.shape[2], s.shape[3]
                if ps < min(sh, sw):
                    patches.append(
                        _unfold(s[:, :, i : sh - (ps - i), j : sw - (ps - j)], ps)
                    )
    for s in scales:
        patches.append(_unfold(s, ps))
    return np.concatenate(patches, axis=1)


# ------------------------------------------------------------------ bass IR
MAX_WAITS = 1


class SplitDrainTileContext(TileContext):
    """This walrus build rejects instructions carrying more than a couple of
    sync waits; split the tail-drain waits across extra SP nops (all before the
    all-engine barrier, so semantics are unchanged)."""

    def _drain_and_barrier(self, tick_clock, wait_clock):
        drain_inst = self.nc.sync.drain()
        wait_clock.add_sem_waits(
            drain_inst.ins, ScopedClock({None: tick_clock.global_clock})
        )
        mi = drain_inst.ins
        waits = list(mi.sync_info.on_wait or []) if mi.sync_info else []
        if len(waits) > MAX_WAITS:
            mi.sync_info.on_wait = waits[:MAX_WAITS]
            rest = waits[MAX_WAITS:]
            while rest:
                chunk, rest = rest[:MAX_WAITS], rest[MAX_WAITS:]
                nop_inst = self.nc.sync.nop(nofuse=True)
                si = nop_inst.ins.sync_info
                if si is None:
                    nop_inst.ins.sync_info = mybir.SyncInfo(
                        on_wait=chunk, on_update=[]
                    )
                else:
                    si.on_wait = (list(si.on_wait) if si.on_wait else []) + chunk

        self.nc.all_engine_barrier()
        assert self.sems is not None
        popped = self.nc._tile_sem_poison_stack.pop()
        assert popped is self._sem_poison
        self.nc.clear_and_free_semaphores(list(self.sems.allocated().values()))
        self.nc.all_engine_barrier()


def _split_waits(nc, max_waits=MAX_WAITS):
    """This walrus build rejects any instruction carrying more than a couple of
    sync waits ("Too many sync wait commands"). Hoist excess waits onto
    same-engine NOPs inserted immediately before the instruction — engines
    dispatch in order, so blocking on the NOP first is equivalent."""
    ctr = 0
    for func in nc.m.functions:
        for bb in func.blocks:
            out = []
            for inst in bb.instructions:
                si = inst.sync_info
                waits = list(si.on_wait) if si is not None and si.on_wait else []
                if len(waits) > max_waits:
                    pre, keep = waits[:-max_waits], waits[-max_waits:]
                    while pre:
                        chunk, pre = pre[:max_waits], pre[max_waits:]
                        out.append(
                            mybir.InstNoOp(
                                name=f"I-waitsplit-{ctr}",
                                engine=inst.engine,
                                sync_info=mybir.SyncInfo(on_wait=chunk, on_update=[]),
                            )
                        )
                        ctr += 1
                    si.on_wait = keep
                out.append(inst)
            bb.instructions[:] = out


def _build_nc():
    f32 = mybir.dt.float32
    nc = bass.Bass(debug=False)
    # Both operand tensors are stored in 4 partition bands of 32 (one per PE
    # row group): band q carries the 28 augmented-feature rows for chunk q so
    # the 4 K=28 matmuls of a tile run concurrently in 4 row groups.
    qT = nc.declare_dram_parameter("qT", [128, NQ], f32, isOutput=False)
    gT = nc.declare_dram_parameter("gT", [128, B * CHUNK], f32, isOutput=False)
    out_val = nc.declare_dram_parameter("out_val", [QT, NTILE], f32, isOutput=True)
    # full 8-slot max_index staging is DMA'd contiguously; host picks slot 0
    out_idx = nc.declare_dram_parameter(
        "out_idx", [QT, NTILE, 8], mybir.dt.uint32, isOutput=True
    )

    with SplitDrainTileContext(nc) as tc:
        with (
            tc.tile_pool(name="weights", bufs=1) as wpool,
            tc.tile_pool(name="psum", bufs=2, space="PSUM") as ppool,
            tc.tile_pool(name="small", bufs=4) as spool,
            tc.tile_pool(name="scratch", bufs=3) as cpool,
            tc.tile_pool(name="outs", bufs=1) as opool,
        ):
            gT_sb = wpool.tile([128, B * CHUNK], f32, tag="gT")
            nc.gpsimd.dma_start(out=gT_sb[:], in_=gT[:])
            # chunk the query DMA into separate tiles so the first matmuls
            # don't wait for the full 2.4MB transfer
            QCHUNK = 6  # tiles per query-DMA piece
            qT_sbs = []
            for j in range(NTILE // QCHUNK):
                qp = wpool.tile([128, QCHUNK * QT], f32, tag=f"qT{j}")
                nc.gpsimd.dma_start(
                    out=qp[:], in_=qT[:, j * QCHUNK * QT : (j + 1) * QCHUNK * QT]
                )
                qT_sbs.append(qp)

            val_sb = opool.tile([QT, NTILE], f32, tag="vals")
            idx_sb = opool.tile([QT, NTILE, 8], mybir.dt.uint32, tag="idxs")

            for t in range(NTILE):
                b = t // (NTILE // B)
                # one PSUM bank per 452-wide chunk (cols 452..511 unused)
                scores = ppool.tile([QT, NCHUNK, BANK], f32, tag="scores")
                qp = qT_sbs[t // QCHUNK]
                qoff = (t % QCHUNK) * QT
                for c in range(NCHUNK):
                    band = 32 * c
                    nc.tensor.matmul(
                        scores[:, c, :CHUNK],
                        qp[band : band + DA, qoff : qoff + QT],
                        gT_sb[band : band + DA, b * CHUNK : (b + 1) * CHUNK],
                        start=True,
                        stop=True,
                        tile_position=(band, 0),
                    )
                # ACT packs the used columns of the 4 banks into one contiguous
                # SBUF buffer (single strided copy); both DVE passes then scan
                # 1808 (not 2048) from SBUF, and the found index IS the
                # core-local candidate id.
                sc = cpool.tile([QT, G_LOC], f32, tag="sc")
                nc.scalar.copy(
                    sc[:].rearrange("p (c k) -> p c k", c=NCHUNK),
                    scores[:, :, :CHUNK],
                )
                # reduce writes its staging column directly; max_index reads the
                # per-partition max through a broadcast AP and writes its own
                # 8-slot staging block (the final DMA picks out slot 0).
                nc.vector.reduce_max(
                    val_sb[:, t : t + 1], sc[:], axis=mybir.AxisListType.X
                )
                nc.vector.max_index(
                    out=idx_sb[:, t, :],
                    in_max=val_sb[:, t : t + 1].to_broadcast([QT, 8]),
                    in_values=sc[:],
                )

            nc.gpsimd.dma_start(out=out_val[:], in_=val_sb[:])
            nc.gpsimd.dma_start(out=out_idx[:], in_=idx_sb[:])
    _split_waits(nc)
    return nc


# ------------------------------------------------------------------- kernel
def _prepare(inp: np.ndarray, tar: np.ndarray):
    """Host prep: patch views, database (bit-identical to the jax reference),
    and the per-core banded input tensors."""
    ipf = _unfold(inp).reshape(B, N, D)
    tpf = _unfold(tar).reshape(B, N, D)
    gf = _g_database(tar).reshape(B, G, D)
    gn = np.einsum("bgd,bgd->bg", gf, gf).astype(np.float32)

    # augmented query matrix, transposed and replicated into the 4 PE
    # row-group bands: [128, 4608]
    s = (tpf + ipf).astype(np.float32)  # [B, N, 27]
    qT = np.zeros((128, NQ), dtype=np.float32)
    for q in range(NCHUNK):
        qT[32 * q : 32 * q + D, :] = s.reshape(NQ, D).T
        qT[32 * q + D, :] = 1.0

    # per-core augmented candidate shards: band q carries chunk q: [128, B*512]
    in_maps = []
    for k in range(NCORES):
        lo = k * G_LOC
        hi = min(lo + G_LOC, G)
        nv = hi - lo
        gTk = np.zeros((128, B * CHUNK), dtype=np.float32)
        ga = np.zeros((B, DA, G_LOC), dtype=np.float32)
        ga[:, D, :] = NEG
        for b in range(B):
            ga[b, :D, :nv] = gf[b, lo:hi, :].T
            ga[b, D, :nv] = -gn[b, lo:hi]
        for q in range(NCHUNK):
            for b in range(B):
                gTk[32 * q : 32 * q + DA, b * CHUNK : (b + 1) * CHUNK] = ga[
                    b, :, q * CHUNK : (q + 1) * CHUNK
                ]
        in_maps.append({"qT": qT, "gT": gTk})
    return in_maps, ipf, gf


def kernel(inp: np.ndarray, tar: np.ndarray):
    global _CACHED_NC
    inp = np.asarray(inp, dtype=np.float32)
    tar = np.asarray(tar, dtype=np.float32)

    in_maps, ipf, gf = _prepare(inp, tar)

    if _CACHED_NC is None:
        _CACHED_NC = _build_nc()
    res = run_bass_kernel_spmd(_CACHED_NC, in_maps, list(range(NCORES)))

    # merge partial argmaxes (ascending core order, strict >, so global
    # first-occurrence tie-breaking is preserved)
    best_val = np.full(NQ, -np.inf, dtype=np.float32)
    best_g = np.zeros(NQ, dtype=np.int64)
    for k in range(NCORES):
        vals = res.results[k]["out_val"].T.reshape(NQ)  # [QT, NTILE] -> [NQ]
        idxs = (
            res.results[k]["out_idx"][:, :, 0].T.reshape(NQ).astype(np.int64)
        )
        upd = vals > best_val
        best_val[upd] = vals[upd]
        best_g[upd] = k * G_LOC + idxs[upd]

    index = best_g.reshape(B, N)
    sel = np.take_along_axis(gf, index[..., None], axis=1)  # [B, N, D]

    loss_p = np.abs(ipf - sel).astype(np.float32)
    loss_img = _fold(loss_p.reshape(B, N, C, PS, PS), C, H, W)
    sel_img = _fold(sel.reshape(B, N, C, PS, PS), C, H, W)
    return np.float32(loss_img.mean(dtype=np.float32)), sel_img



# revision 27
# speedup vs baseline: 1.6703x; 1.0076x over previous
"""Trainium2 Bass kernel for nn_BB_loss_80298708566608 (retrieval_knn).

Reference computation: for each of B*N query patches (3x3x3 = 27-dim), find the
nearest candidate patch among G=14460 database patches (built from `tar` at 3
scales with shifted grids), under l = 0.5*||tpf-g||^2 + 0.5*||ipf-g||^2.
Since the query-norm term is constant in g, argmin_g l == argmax_g score with
  score(q, g) = (tpf_q + ipf_q) . g - ||g||^2
which is one [4608, 28] x [28, G] matmul (augmented with a constant-1 column
against -||g||^2) fused with an argmax over the candidate axis.

Device strategy (8 NeuronCores, candidate-sharded):
  - core k owns candidates [k*1808, k*1808+1808) (last core: 1804 real + 4
    filler rows whose augmented column is -1e30 so they never win), split into
    4 chunks of 452.
  - every core processes all 36 query tiles of 128 queries. Per tile: the 4
    K=28 fp32 matmuls run CONCURRENTLY in the 4 PE row groups (operands stored
    in 32-partition bands, tile_position=(32q,0)), each writing one PSUM bank;
    ACT packs the 4x452 used columns into one contiguous SBUF row; DVE then
    does reduce_max + max_index over [128,1808] (max_index returns
    first-occurrence indices = jnp.argmin tie semantics).
  - outputs per core: local max value + local argmax per query; the host merges
    the 8 partial argmaxes (ascending core order, strict >, preserving global
    first-tie-break), gathers the selected patches, and computes the L1 loss
    and the folded image.

The patch database is built on the host in numpy; it is bit-identical to the
reference's jax implementation (validated).
"""

import numpy as np

import concourse.bass as bass
import concourse.mybir as mybir
from concourse.tile import TileContext
from concourse.bass_utils import run_bass_kernel_spmd
from concourse.vector_clock import ScopedClock

# ---------------------------------------------------------------- constants
PS = 3
A_CUBIC = -0.75
B, C, H, W = 2, 3, 144, 144
N = (H // PS) * (W // PS)  # 2304 queries per batch
D = C * PS * PS  # 27
DA = D + 1  # augmented with constant-1 column
G = 14460  # database size per batch
NCORES = 8
G_LOC = 1808  # candidates per core (last core: 1804 valid)
NCHUNK = 4
CHUNK = G_LOC // NCHUNK  # 452 — matmul free dim per PSUM bank (bank holds 512)
BANK = 512
NQ = B * N  # 4608 total queries
QT = 128  # queries per tile
NTILE = NQ // QT  # 36
NEG = -1.0e30

_CACHED_NC = None


# ------------------------------------------------------------- database build
def _cubic_w(t):
    at = np.abs(t)
    w1 = ((A_CUBIC + 2.0) * at - (A_CUBIC + 3.0)) * at * at + 1.0
    w2 = A_CUBIC * (((at - 5.0) * at + 8.0) * at - 4.0)
    return np.where(at <= 1.0, w1, np.where(at < 2.0, w2, 0.0)).astype(t.dtype)


def _resize_axis(x, out_size, axis):
    in_size = x.shape[axis]
    scale = (in_size - 1) / (out_size - 1)
    coords = np.arange(out_size, dtype=x.dtype) * np.asarray(scale, x.dtype)
    i0 = np.floor(coords)
    frac = coords - i0
    offs = np.arange(-1, 3)
    idx = np.clip(i0.astype(np.int32)[:, None] + offs[None, :], 0, in_size - 1)
    w = _cubic_w(frac[:, None] - offs[None, :].astype(x.dtype))
    xm = np.moveaxis(x, axis, -1)
    vals = xm[..., idx]
    out = np.einsum("...ot,ot->...o", vals, w)
    return np.moveaxis(out, -1, axis).astype(x.dtype)


def _bicubic(x, out_h, out_w):
    return _resize_axis(_resize_axis(x, out_h, 2), out_w, 3)


def _unfold(x, ps=PS):
    b, c, h, w = x.shape
    x = x.reshape(b, c, h // ps, ps, w // ps, ps)
    x = x.transpose(0, 2, 4, 1, 3, 5)
    return x.reshape(b, -1, c, ps, ps)


def _fold(p, c, h, w, ps=PS):
    b = p.shape[0]
    x = p.reshape(b, h // ps, w // ps, c, ps, ps)
    x = x.transpose(0, 3, 1, 4, 2, 5)
    return x.reshape(b, c, h, w)


def _g_database(tar, ps=PS):
    b, c, h, w = tar.shape
    scales = [tar, _bicubic(tar, h // 2, w // 2), _bicubic(tar, h // 4, w // 4)]
    patches = []
    for i in range(1, ps):
        for j in range(1, ps):
            for s in scales:
                sh, sw = s.shape[2], s.shape[3]
                if ps < min(sh, sw):
                    patches.append(
                        _unfold(s[:, :, i : sh - (ps - i), j : sw - (ps - j)], ps)
                    )
    for s in scales:
        patches.append(_unfold(s, ps))
    return np.concatenate(patches, axis=1)


# ------------------------------------------------------------------ bass IR
MAX_WAITS = 1


class SplitDrainTileContext(TileContext):
    """This walrus build rejects instructions carrying more than a couple of
    sync waits; split the tail-drain waits across extra SP nops (all before the
    all-engine barrier, so semantics are unchanged)."""

    def _drain_and_barrier(self, tick_clock, wait_clock):
        drain_inst = self.nc.sync.drain()
        wait_clock.add_sem_waits(
            drain_inst.ins, ScopedClock({None: tick_clock.global_clock})
        )
        mi = drain_inst.ins
        waits = list(mi.sync_info.on_wait or []) if mi.sync_info else []
        if len(waits) > MAX_WAITS:
            mi.sync_info.on_wait = waits[:MAX_WAITS]
            rest = waits[MAX_WAITS:]
            while rest:
                chunk, rest = rest[:MAX_WAITS], rest[MAX_WAITS:]
                nop_inst = self.nc.sync.nop(nofuse=True)
                si = nop_inst.ins.sync_info
                if si is None:
                    nop_inst.ins.sync_info = mybir.SyncInfo(
                        on_wait=chunk, on_update=[]
                    )
                else:
                    si.on_wait = (list(si.on_wait) if si.on_wait else []) + chunk

        self.nc.all_engine_barrier()
        assert self.sems is not None
        popped = self.nc._tile_sem_poison_stack.pop()
        assert popped is self._sem_poison
        self.nc.clear_and_free_semaphores(list(self.sems.allocated().values()))
        self.nc.all_engine_barrier()


def _split_waits(nc, max_waits=MAX_WAITS):
    """This walrus build rejects any instruction carrying more than a couple of
    sync waits ("Too many sync wait commands"). Hoist excess waits onto
    same-engine NOPs inserted immediately before the instruction — engines
    dispatch in order, so blocking on the NOP first is equivalent."""
    ctr = 0
    for func in nc.m.functions:
        for bb in func.blocks:
            out = []
            for inst in bb.instructions:
                si = inst.sync_info
                waits = list(si.on_wait) if si is not None and si.on_wait else []
                if len(waits) > max_waits:
                    pre, keep = waits[:-max_waits], waits[-max_waits:]
                    while pre:
                        chunk, pre = pre[:max_waits], pre[max_waits:]
                        out.append(
                            mybir.InstNoOp(
                                name=f"I-waitsplit-{ctr}",
                                engine=inst.engine,
                                sync_info=mybir.SyncInfo(on_wait=chunk, on_update=[]),
                            )
                        )
                        ctr += 1
                    si.on_wait = keep
                out.append(inst)
            bb.instructions[:] = out


def _build_nc():
    f32 = mybir.dt.float32
    nc = bass.Bass(debug=False)
    # Both operand tensors are stored in 4 partition bands of 32 (one per PE
    # row group): band q carries the 28 augmented-feature rows for chunk q so
    # the 4 K=28 matmuls of a tile run concurrently in 4 row groups.
    qT = nc.declare_dram_parameter("qT", [128, NQ], f32, isOutput=False)
    gT = nc.declare_dram_parameter("gT", [128, B * CHUNK], f32, isOutput=False)
    out_val = nc.declare_dram_parameter("out_val", [QT, NTILE], f32, isOutput=True)
    # full 8-slot max_index staging is DMA'd contiguously; host picks slot 0
    out_idx = nc.declare_dram_parameter(
        "out_idx", [QT, NTILE, 8], mybir.dt.uint32, isOutput=True
    )

    with SplitDrainTileContext(nc) as tc:
        with (
            tc.tile_pool(name="weights", bufs=1) as wpool,
            tc.tile_pool(name="psum", bufs=2, space="PSUM") as ppool,
            tc.tile_pool(name="small", bufs=4) as spool,
            tc.tile_pool(name="scratch", bufs=3) as cpool,
            tc.tile_pool(name="outs", bufs=1) as opool,
        ):
            # chunk the query DMA into separate tiles, smallest pieces first,
            # so the first matmuls don't wait for the full 2.4MB transfer
            pieces = [2, 4, 6, 6, 6, 6, 6]  # tiles per query-DMA piece
            gT_sb = wpool.tile([128, B * CHUNK], f32, tag="gT")
            qT_sbs = []  # tile index -> (piece tile, column offset)
            start = 0
            for j, ntp in enumerate(pieces):
                qp = wpool.tile([128, ntp * QT], f32, tag=f"qT{j}")
                nc.gpsimd.dma_start(
                    out=qp[:], in_=qT[:, start * QT : (start + ntp) * QT]
                )
                if j == 0:
                    # batch-0 candidates right after the first query piece
                    nc.gpsimd.dma_start(out=gT_sb[:, :CHUNK], in_=gT[:, :CHUNK])
                    nc.gpsimd.dma_start(out=gT_sb[:, CHUNK:], in_=gT[:, CHUNK:])
                for t in range(start, start + ntp):
                    qT_sbs.append((qp, (t - start) * QT))
                start += ntp

            val_sb = opool.tile([QT, NTILE], f32, tag="vals")
            idx_sb = opool.tile([QT, NTILE, 8], mybir.dt.uint32, tag="idxs")

            for t in range(NTILE):
                b = t // (NTILE // B)
                # one PSUM bank per 452-wide chunk (cols 452..511 unused)
                scores = ppool.tile([QT, NCHUNK, BANK], f32, tag="scores")
                qp, qoff = qT_sbs[t]
                for c in range(NCHUNK):
                    band = 32 * c
                    nc.tensor.matmul(
                        scores[:, c, :CHUNK],
                        qp[band : band + DA, qoff : qoff + QT],
                        gT_sb[band : band + DA, b * CHUNK : (b + 1) * CHUNK],
                        start=True,
                        stop=True,
                        tile_position=(band, 0),
                    )
                # ACT packs the used columns of the 4 banks into one contiguous
                # SBUF buffer (single strided copy); both DVE passes then scan
                # 1808 (not 2048) from SBUF, and the found index IS the
                # core-local candidate id.
                sc = cpool.tile([QT, G_LOC], f32, tag="sc")
                nc.scalar.copy(
                    sc[:].rearrange("p (c k) -> p c k", c=NCHUNK),
                    scores[:, :, :CHUNK],
                )
                # reduce writes its staging column directly; max_index reads the
                # per-partition max through a broadcast AP and writes its own
                # 8-slot staging block (the final DMA picks out slot 0).
                nc.vector.reduce_max(
                    val_sb[:, t : t + 1], sc[:], axis=mybir.AxisListType.X
                )
                nc.vector.max_index(
                    out=idx_sb[:, t, :],
                    in_max=val_sb[:, t : t + 1].to_broadcast([QT, 8]),
                    in_values=sc[:],
                )

                if t == NTILE // 2 - 1:
                    # first-half outputs leave while the second half computes
                    h = NTILE // 2
                    nc.gpsimd.dma_start(
                        out=out_val[:, :h], in_=val_sb[:, :h]
                    )
                    nc.gpsimd.dma_start(
                        out=out_idx[:, :h, :], in_=idx_sb[:, :h, :]
                    )

            h = NTILE // 2
            nc.gpsimd.dma_start(out=out_val[:, h:], in_=val_sb[:, h:])
            nc.gpsimd.dma_start(out=out_idx[:, h:, :], in_=idx_sb[:, h:, :])
    _split_waits(nc)
    return nc


# ------------------------------------------------------------------- kernel
def _prepare(inp: np.ndarray, tar: np.ndarray):
    """Host prep: patch views, database (bit-identical to the jax reference),
    and the per-core banded input tensors."""
    ipf = _unfold(inp).reshape(B, N, D)
    tpf = _unfold(tar).reshape(B, N, D)
    gf = _g_database(tar).reshape(B, G, D)
    gn = np.einsum("bgd,bgd->bg", gf, gf).astype(np.float32)

    # augmented query matrix, transposed and replicated into the 4 PE
    # row-group bands: [128, 4608]
    s = (tpf + ipf).astype(np.float32)  # [B, N, 27]
    qT = np.zeros((128, NQ), dtype=np.float32)
    for q in range(NCHUNK):
        qT[32 * q : 32 * q + D, :] = s.reshape(NQ, D).T
        qT[32 * q + D, :] = 1.0

    # per-core augmented candidate shards: band q carries chunk q: [128, B*512]
    in_maps = []
    for k in range(NCORES):
        lo = k * G_LOC
        hi = min(lo + G_LOC, G)
        nv = hi - lo
        gTk = np.zeros((128, B * CHUNK), dtype=np.float32)
        ga = np.zeros((B, DA, G_LOC), dtype=np.float32)
        ga[:, D, :] = NEG
        for b in range(B):
            ga[b, :D, :nv] = gf[b, lo:hi, :].T
            ga[b, D, :nv] = -gn[b, lo:hi]
        for q in range(NCHUNK):
            for b in range(B):
                gTk[32 * q : 32 * q + DA, b * CHUNK : (b + 1) * CHUNK] = ga[
                    b, :, q * CHUNK : (q + 1) * CHUNK
                ]
        in_maps.append({"qT": qT, "gT": gTk})
    return in_maps, ipf, gf


def kernel(inp: np.ndarray, tar: np.ndarray):
    global _CACHED_NC
    inp = np.asarray(inp, dtype=np.float32)
    tar = np.asarray(tar, dtype=np.float32)

    in_maps, ipf, gf = _prepare(inp, tar)

    if _CACHED_NC is None:
        _CACHED_NC = _build_nc()
    res = run_bass_kernel_spmd(_CACHED_NC, in_maps, list(range(NCORES)))

    # merge partial argmaxes (ascending core order, strict >, so global
    # first-occurrence tie-breaking is preserved)
    best_val = np.full(NQ, -np.inf, dtype=np.float32)
    best_g = np.zeros(NQ, dtype=np.int64)
    for k in range(NCORES):
        vals = res.results[k]["out_val"].T.reshape(NQ)  # [QT, NTILE] -> [NQ]
        idxs = (
            res.results[k]["out_idx"][:, :, 0].T.reshape(NQ).astype(np.int64)
        )
        upd = vals > best_val
        best_val[upd] = vals[upd]
        best_g[upd] = k * G_LOC + idxs[upd]

    index = best_g.reshape(B, N)
    sel = np.take_along_axis(gf, index[..., None], axis=1)  # [B, N, D]

    loss_p = np.abs(ipf - sel).astype(np.float32)
    loss_img = _fold(loss_p.reshape(B, N, C, PS, PS), C, H, W)
    sel_img = _fold(sel.reshape(B, N, C, PS, PS), C, H, W)
    return np.float32(loss_img.mean(dtype=np.float32)), sel_img

